# revision 3
# baseline (speedup 1.0000x reference)
"""Trainium2 Bass kernel for the CSTR gated-estimator trajectory-cost problem.

Math reformulation (vs the per-step reference scan):
  The RK4 step with linear plant dynamics reduces to an affine update
      x_{t+1} = P x_t + q*u_t*[1,1] + c + w_t,   P = I + H*A
  which diagonalizes in the (+/-) basis yp = x1+x2, ym = x1-x2:
      yp_{t+1} = 0.99*yp_t + (2q*u_t + ccp + wp_t)
      ym_{t+1} = 0.97*ym_t + (ccm + wm_t)            (control-free)
  The estimator gate recurrence likewise stays diagonal in that basis:
      etap_t = (1-d_t)*etap_{t-1} + d_t*yp_t        (same for etam/ym)
  with d_t = sigmoid(phi_t), phi a quadratic form in
  v = (yp_t, ym_t, etap_{t-1}, etam_{t-1}).

  Given the gate sequence d, every recurrence is a first-order linear
  scan -> one hardware tensor_tensor_scan instruction over the whole
  time axis.  The nonlinear gate is resolved by Picard fixed-point
  iteration (gate feedback is weak: contraction ~0.1/iter), which
  converges to the f32 rounding floor in 3 iterations after a
  constant-gate warm start.

Layout: batch across partitions+chunks (8 cores x 8 chunks x 128 samples),
time along the free dimension. All sequential-in-time work is inside scan
instructions; everything else is full-width elementwise DVE/ACT/GPSIMD ops.
"""

import numpy as np

import concourse.bacc as bacc
import concourse.bass as bass
import concourse.mybir as mybir
import concourse.tile as tile
from concourse import bass_utils

dt = mybir.dt
F32 = dt.float32
AF = mybir.ActivationFunctionType
OP = mybir.AluOpType
AX = mybir.AxisListType

B_TOTAL = 8192
N_CORES = 8
T = 2048
B_CORE = B_TOTAL // N_CORES          # 1024
N_CHUNKS = B_CORE // 128             # 8
H = 0.01
R_COST = 0.1
N_FULL_ITERS = 3                     # full Picard iterations after warm start


def _host_consts(K, L, M, Mo):
    """All scalar constants, computed in f64 then truncated to f32 immediates."""
    k1 = float(np.asarray(K)[0, 0]); k2 = float(np.asarray(K)[0, 1])
    kp, km = (k1 + k2) / 2.0, (k1 - k2) / 2.0
    c = {}
    c["lamp"] = 1.0 - 2 * H + H       # 0.99
    c["lamm"] = 1.0 - 2 * H - H       # 0.97
    c1, c2 = H * H / 2.0, -H * H
    c["ccp"] = c1 + c2
    c["ccm"] = c1 - c2
    c["ap_u"] = 2.0 * H * kp          # u-feedback coefficients into yp input
    c["am_u"] = 2.0 * H * km
    S = (np.asarray(L, np.float64) + np.asarray(L, np.float64).T) / 2.0
    A = np.array([[.5, .5, 0, 0], [.5, -.5, 0, 0],
                  [0, 0, .5, .5], [0, 0, .5, -.5]], np.float64)
    St = A.T @ S @ A                  # phi = v' St v + bt.v + mo
    bt = A.T @ np.asarray(M, np.float64)[0]
    c["St"] = St
    c["bt"] = bt
    c["mo"] = float(np.asarray(Mo)[0, 0])
    c["dbar"] = float(1.0 / (1.0 + np.exp(-c["mo"])))
    c["alpha_p"] = 0.5 + R_COST * kp * kp
    c["alpha_m"] = 0.5 + R_COST * km * km
    c["alpha_x"] = 2.0 * R_COST * kp * km
    c["sc0"] = 2.0 + R_COST * k1 * k1   # t=0 stage cost (x0=(1,0), delta_0=1)
    return c


def _build(c, n_chunks=N_CHUNKS, n_full=N_FULL_ITERS,
           use_bcast=False, use_accum=False):
    nc = bacc.Bacc("TRN2", target_bir_lowering=False, debug=False,
                   num_devices=N_CORES)
    w_in = nc.dram_tensor("w", [n_chunks * 128, 2, T], F32,
                          kind="ExternalInput").ap()
    j_out = nc.dram_tensor("j", [n_chunks * 128, 1], F32,
                           kind="ExternalOutput").ap()

    St = c["St"]; bt = c["bt"]
    s11, s22, s33, s44 = (float(St[i, i]) for i in range(4))
    s12, s13, s14 = float(2 * St[0, 1]), float(2 * St[0, 2]), float(2 * St[0, 3])
    s23, s24 = float(2 * St[1, 2]), float(2 * St[1, 3])
    s34 = float(2 * St[2, 3])
    b1, b2, b3, b4 = (float(bt[i]) for i in range(4))
    sgn = [1.0 if s >= 0 else -1.0 for s in (s11, s22, s33, s44)]
    sq_scale = [float(np.sqrt(abs(s))) for s in (s11, s22, s33, s44)]

    with tile.TileContext(nc) as tc:
        with (
            tc.tile_pool(name="const", bufs=1) as cpool,
            tc.tile_pool(name="wload", bufs=2) as wpool,
            tc.tile_pool(name="work", bufs=1) as wk,
            tc.tile_pool(name="sq", bufs=2) as sqpool,
            tc.tile_pool(name="tiny", bufs=2) as tiny,
        ):
            mob = cpool.tile([128, 1], F32, tag="mob")
            nc.gpsimd.memset(mob[:, :], c["mo"])
            if use_bcast:
                lamp1 = cpool.tile([128, 1], F32, tag="lamp1")
                lamm1 = cpool.tile([128, 1], F32, tag="lamm1")
                nc.gpsimd.memset(lamp1[:, :], c["lamp"])
                nc.gpsimd.memset(lamm1[:, :], c["lamm"])
                lamp_b = lamp1[:, :].to_broadcast((128, T))
                lamm_b = lamm1[:, :].to_broadcast((128, T))
            else:
                lampf = cpool.tile([128, T], F32, tag="lampf")
                lammf = cpool.tile([128, T], F32, tag="lammf")
                nc.gpsimd.memset(lampf[:, :], c["lamp"])
                nc.gpsimd.memset(lammf[:, :], c["lamm"])
                lamp_b = lampf[:, :]
                lamm_b = lammf[:, :]

            for ch in range(n_chunks):
                rows = slice(ch * 128, (ch + 1) * 128)
                W1 = wpool.tile([128, T], F32, tag="W1")
                W2 = wpool.tile([128, T], F32, tag="W2")
                nc.sync.dma_start(W1[:, :], w_in[rows, 0, :])
                nc.sync.dma_start(W2[:, :], w_in[rows, 1, :])

                WPC = wk.tile([128, T], F32, tag="WPC")
                D1M = wk.tile([128, T], F32, tag="DM")   # shares slot with DM1
                nc.gpsimd.tensor_tensor(WPC[:, :], W1[:, :], W2[:, :], OP.add)
                nc.gpsimd.tensor_scalar(WPC[:, :], WPC[:, :], c["ccp"], None, OP.add)
                nc.gpsimd.tensor_tensor(D1M[:, :], W1[:, :], W2[:, :], OP.subtract)
                nc.gpsimd.tensor_scalar(D1M[:, :], D1M[:, :], c["ccm"], None, OP.add)

                YM = wk.tile([128, T + 1], F32, tag="YM")
                YP = wk.tile([128, T + 1], F32, tag="YP")
                nc.gpsimd.memset(YM[:, 0:1], 1.0)
                nc.gpsimd.memset(YP[:, 0:1], 1.0)
                nc.vector.tensor_tensor_scan(
                    YM[:, 1:T + 1], lamm_b, D1M[:, :], 1.0, OP.mult, OP.add)

                ETAP = wk.tile([128, T], F32, tag="ETAP")
                ETAM = wk.tile([128, T], F32, tag="ETAM")
                D = wk.tile([128, T], F32, tag="D")
                D1P = wk.tile([128, T], F32, tag="D1P")
                D1Me = wk.tile([128, T], F32, tag="D1Me")
                EB = wk.tile([128, T], F32, tag="EB")
                PH = wk.tile([128, T - 1], F32, tag="PH")
                PB = wk.tile([128, T - 1], F32, tag="PB")
                G1 = wk.tile([128, T - 1], F32, tag="G1")
                G2 = wk.tile([128, T - 1], F32, tag="G2")
                G3 = wk.tile([128, T - 1], F32, tag="G3")

                AD = tiny.tile([128, 1], F32, tag="AD")
                A1 = tiny.tile([128, 1], F32, tag="A1")
                A2 = tiny.tile([128, 1], F32, tag="A2")
                A3 = tiny.tile([128, 1], F32, tag="A3")
                TA = tiny.tile([128, 1], F32, tag="TA")
                TB = tiny.tile([128, 1], F32, tag="TB")
                JT = tiny.tile([128, 1], F32, tag="JT")

                n_iters = n_full + 1
                for it in range(n_iters):
                    final = (it == n_iters - 1)
                    # --- input sequence e for the yp scan ---
                    if it == 0:
                        e_ap = WPC[:, :]
                    else:
                        nc.vector.scalar_tensor_tensor(
                            EB[:, :], ETAP[:, :], c["ap_u"], WPC[:, :],
                            OP.mult, OP.add)
                        nc.vector.scalar_tensor_tensor(
                            EB[:, :], ETAM[:, :], c["am_u"], EB[:, :],
                            OP.mult, OP.add)
                        e_ap = EB[:, :]
                    nc.vector.tensor_tensor_scan(
                        YP[:, 1:T + 1], lamp_b, e_ap, 1.0, OP.mult, OP.add)

                    # --- gate sequence ---
                    if it == 0:
                        # warm start: constant gate dbar (no phi/sigmoid)
                        nc.gpsimd.memset(D[:, :], c["dbar"])
                        nc.gpsimd.memset(D[:, 0:1], 0.0)
                    else:
                        v1 = YP[:, 1:T]
                        v2 = YM[:, 1:T]
                        v3 = ETAP[:, 0:T - 1]
                        v4 = ETAM[:, 0:T - 1]
                        # g-vectors (upper-triangular factorization of phi)
                        nc.vector.tensor_scalar(G1[:, :], v2, s12, b1, OP.mult, OP.add)
                        nc.vector.scalar_tensor_tensor(
                            G1[:, :], v3, s13, G1[:, :], OP.mult, OP.add)
                        nc.vector.scalar_tensor_tensor(
                            G1[:, :], v4, s14, G1[:, :], OP.mult, OP.add)
                        nc.vector.tensor_scalar(G2[:, :], v3, s23, b2, OP.mult, OP.add)
                        nc.vector.scalar_tensor_tensor(
                            G2[:, :], v4, s24, G2[:, :], OP.mult, OP.add)
                        nc.vector.tensor_scalar(G3[:, :], v4, s34, b3, OP.mult, OP.add)
                        # diagonal squares on ACT
                        Rts = []
                        for vi, sc in zip((v1, v2, v3, v4), sq_scale):
                            Rt = sqpool.tile([128, T - 1], F32, tag="R")
                            nc.scalar.activation(Rt[:, :], vi, AF.Square, scale=sc)
                            Rts.append(Rt)
                        # phi accumulation
                        nc.vector.tensor_tensor(PH[:, :], v1, G1[:, :], OP.mult)
                        nc.vector.tensor_tensor(PB[:, :], v2, G2[:, :], OP.mult)
                        nc.vector.tensor_tensor(PH[:, :], PH[:, :], PB[:, :], OP.add)
                        nc.vector.tensor_tensor(PB[:, :], v3, G3[:, :], OP.mult)
                        nc.vector.scalar_tensor_tensor(
                            PB[:, :], v4, b4, PB[:, :], OP.mult, OP.add)
                        nc.vector.tensor_tensor(PH[:, :], PH[:, :], PB[:, :], OP.add)
                        for Rt, sg in zip(Rts, sgn):
                            nc.vector.tensor_tensor(
                                PH[:, :], PH[:, :], Rt[:, :],
                                OP.add if sg > 0 else OP.subtract)
                        if final and use_accum:
                            nc.scalar.activation(D[:, 1:T], PH[:, :], AF.Sigmoid,
                                                 bias=mob[:, 0:1], scale=1.0,
                                                 accum_out=AD[:, :])
                        else:
                            nc.scalar.activation(D[:, 1:T], PH[:, :], AF.Sigmoid,
                                                 bias=mob[:, 0:1], scale=1.0)

                    if not final:
                        # eta scans for the next iteration
                        DM1 = wk.tile([128, T], F32, tag="DM")
                        nc.scalar.activation(DM1[:, :], D[:, :], AF.Identity,
                                             bias=1.0, scale=-1.0)
                        nc.gpsimd.tensor_tensor(D1P[:, :], D[:, :], YP[:, 0:T], OP.mult)
                        nc.gpsimd.tensor_tensor(D1Me[:, :], D[:, :], YM[:, 0:T], OP.mult)
                        nc.vector.tensor_tensor_scan(
                            ETAP[:, :], DM1[:, :], D1P[:, :], 1.0, OP.mult, OP.add)
                        nc.vector.tensor_tensor_scan(
                            ETAM[:, :], DM1[:, :], D1Me[:, :], 1.0, OP.mult, OP.add)

                # --- cost assembly (uses last iteration's YP, D, AD) ---
                yps = YP[:, 1:T - 1]     # t = 1..T-2
                yms = YM[:, 1:T - 1]
                SCR = PB[:, 0:T - 2]     # scratch full-width output
                if use_accum:
                    nc.vector.scalar_tensor_tensor(
                        SCR, yps, c["alpha_p"], yps, OP.mult, OP.mult,
                        accum_out=A1[:, :])
                    nc.vector.scalar_tensor_tensor(
                        SCR, yms, c["alpha_m"], yms, OP.mult, OP.mult,
                        accum_out=A2[:, :])
                    nc.vector.scalar_tensor_tensor(
                        SCR, yps, c["alpha_x"], yms, OP.mult, OP.mult,
                        accum_out=A3[:, :])
                else:
                    nc.vector.scalar_tensor_tensor(
                        SCR, yps, c["alpha_p"], yps, OP.mult, OP.mult)
                    nc.vector.tensor_reduce(A1[:, :], SCR, AX.X, OP.add)
                    nc.vector.scalar_tensor_tensor(
                        SCR, yms, c["alpha_m"], yms, OP.mult, OP.mult)
                    nc.vector.tensor_reduce(A2[:, :], SCR, AX.X, OP.add)
                    nc.vector.scalar_tensor_tensor(
                        SCR, yps, c["alpha_x"], yms, OP.mult, OP.mult)
                    nc.vector.tensor_reduce(A3[:, :], SCR, AX.X, OP.add)
                    nc.vector.tensor_reduce(AD[:, :], D[:, 1:T], AX.X, OP.add)
                nc.vector.scalar_tensor_tensor(
                    TA[:, :], YP[:, T:T + 1], 5.0, YP[:, T:T + 1], OP.mult, OP.mult)
                nc.vector.scalar_tensor_tensor(
                    TB[:, :], YM[:, T:T + 1], 5.0, YM[:, T:T + 1], OP.mult, OP.mult)
                nc.vector.tensor_tensor(JT[:, :], A1[:, :], A2[:, :], OP.add)
                nc.vector.tensor_tensor(JT[:, :], JT[:, :], A3[:, :], OP.add)
                # AD sums delta_t over t=1..T-1; cost wants t=1..T-2
                nc.vector.tensor_tensor(JT[:, :], JT[:, :], AD[:, :], OP.add)
                nc.vector.tensor_tensor(JT[:, :], JT[:, :], D[:, T - 1:T], OP.subtract)
                nc.vector.tensor_tensor(JT[:, :], JT[:, :], TA[:, :], OP.add)
                nc.vector.tensor_tensor(JT[:, :], JT[:, :], TB[:, :], OP.add)
                nc.vector.tensor_scalar(JT[:, :], JT[:, :], c["sc0"], None, OP.add)
                nc.sync.dma_start(j_out[rows, :], JT[:, :])

    nc.compile()
    return nc


def kernel(w, K, L, M, Mo, trace=False, n_full=N_FULL_ITERS,
           use_bcast=False, use_accum=False):
    w = np.ascontiguousarray(np.asarray(w, np.float32))
    assert w.shape == (B_TOTAL, 2, T), w.shape
    c = _host_consts(K, L, M, Mo)
    nc = _build(c, n_full=n_full, use_bcast=use_bcast, use_accum=use_accum)
    in_maps = [{"w": w[i * B_CORE:(i + 1) * B_CORE]} for i in range(N_CORES)]
    res = bass_utils.run_bass_kernel_spmd(
        nc, in_maps, list(range(N_CORES)), trace=trace)
    out = np.concatenate([res.results[i]["j"].reshape(-1)
                          for i in range(N_CORES)])
    if trace:
        kernel.last_exec_time_ns = res.exec_time_ns
        kernel.last_results = res
    return out.astype(np.float32)


# revision 8
# speedup vs baseline: 3.9169x; 3.9169x over previous
"""Trainium2 Bass kernel for the CSTR gated-estimator trajectory-cost problem.

Reformulation vs the reference's 2048-step sequential scan:

1. The RK4 step with linear plant dynamics is an affine update
       x_{t+1} = P x_t + q*u_t*[1,1] + c + w_t,   P = I + H*A,
   which diagonalizes in the +/- basis yp = x1+x2, ym = x1-x2:
       yp_{t+1} = 0.99*yp_t + (2q*u_t + ccp + wp_t)
       ym_{t+1} = 0.97*ym_t + (ccm + wm_t)        (control-free chain)
   The gated-estimator recurrence stays diagonal in the same basis:
       etap_t = (1-d_t)*etap_{t-1} + d_t*yp_t     (ditto etam/ym)
   with gate d_t = sigmoid(phi_t), phi_t a quadratic form in
   v_t = (yp_t, ym_t, etap_{t-1}, etam_{t-1}).

2. Every recurrence above is a first-order linear scan along time ->
   one hardware `tensor_tensor_scan` instruction per sequence.

3. The nonlinear gate is resolved by one Picard correction on top of an
   EXACT warm start: freezing the gate at dbar = sigmoid(Mo), the coupled
   (yp, etap) system is linear time-invariant; its 2x2 transition matrix
   is eigendecomposed on the host, so the warm-start trajectories
   (including the full control feedback) come from two more scans.
   One full Picard iteration then restores the time-varying gate;
   the residual is below the f32 rounding floor of the reference
   (validated: max rel err ~2.4e-6, identical to 3 iterations).

4. phi is evaluated per time-step via an LDL^T factorization:
   phi = sum_i d_i (m_i + g_i)^2 + const with unit-triangular m-chains
   (6 scalar_tensor_tensor ops), squares+shifts on the Scalar engine
   (Square activation with per-partition bias), and the constant folded
   into the sigmoid bias. The cost quadratic is likewise
   complete-squared so its time-sums come free from Square+accum_out.

Layout: pure data parallel. 8192 samples = 8 cores x 8 chunks x 128
partitions; time (2048) along the free dimension. HBM traffic is w read
exactly once. The only sequential-in-time work is inside scan
instructions; everything else is full-width elementwise DVE/ACT/GPSIMD.
"""

import numpy as np

import concourse.bacc as bacc
import concourse.bass as bass
import concourse.mybir as mybir
import concourse.tile as tile
from concourse import bass_utils

dt = mybir.dt
F32 = dt.float32
AF = mybir.ActivationFunctionType
OP = mybir.AluOpType
AX = mybir.AxisListType

B_TOTAL = 8192
N_CORES = 8
T = 2048
B_CORE = B_TOTAL // N_CORES          # 1024
N_CHUNKS = B_CORE // 128             # 8
H = 0.01
R_COST = 0.1


def _host_consts(K, L, M, Mo, n_full=1):
    """All scalar constants, computed in f64, emitted as f32 immediates."""
    k1 = float(np.asarray(K)[0, 0]); k2 = float(np.asarray(K)[0, 1])
    kp, km = (k1 + k2) / 2.0, (k1 - k2) / 2.0
    c = {"n_full": n_full}
    lamp, lamm = 1.0 - H, 1.0 - 3 * H          # 0.99, 0.97
    c["lamp"], c["lamm"] = lamp, lamm
    c1, c2 = H * H / 2.0, -H * H
    c["ccp"], c["ccm"] = c1 + c2, c1 - c2
    a_u, b_u = 2.0 * H * kp, 2.0 * H * km       # u-feedback coeffs (yp input)
    mo = float(np.asarray(Mo)[0, 0])
    dbar = 1.0 / (1.0 + np.exp(-mo))
    c["mo"], c["dbar"] = mo, dbar

    # --- exact-LTI warm start: s_t = (yp_t, etap_{t-1}) ---
    A2 = np.array([[lamp + a_u * dbar, a_u * (1 - dbar)],
                   [dbar, 1 - dbar]], np.float64)
    eigval, V = np.linalg.eig(A2)
    assert np.abs(eigval.imag).max() < 1e-12, "complex warm-start eigs"
    eigval = eigval.real; V = V.real
    Vi = np.linalg.inv(V)
    z0 = Vi @ np.array([1.0, 1.0])
    c["mu1"], c["mu2"] = float(eigval[0]), float(eigval[1])
    c["z01"], c["z02"] = float(z0[0]), float(z0[1])
    # eta-tilde scaling: work with etaps = a_u*etap, etams = b_u*etam so the
    # E-chain is plain tensor-tensor adds. (Guard b_u ~ 0.)
    bs = b_u if abs(b_u) > 1e-12 else 1e-12
    as_ = a_u if abs(a_u) > 1e-12 else 1e-12
    c["a_u"], c["b_u"] = as_, bs
    # etaps_t = as_*(V10 z1_t+1 + V11 z2_t+1): STT(Z1, rat_p, Z2) * (as_*V11)
    assert abs(V[1, 1]) > 1e-8 and abs(V[1, 0]) > 1e-8
    c["rat_p"] = float(V[1, 0] / V[1, 1])
    c["scl_p"] = float(as_ * V[1, 1])
    # r_t = b_u*etam + (ccp + wp) = etams + wpc ; z-scan d1 scales:
    c["dz1"], c["dz2"] = float(Vi[0, 0]), float(Vi[1, 0])
    # etams-chain: d1 = (bs*dbar)*ym, d0 = 1-dbar
    c["etams_d1s"] = float(bs * dbar)
    c["etams_init"] = float(bs * 1.0)

    # --- phi quadratic in v = (yp, ym, etaps, etams) (scaled vars) ---
    S = (np.asarray(L, np.float64) + np.asarray(L, np.float64).T) / 2.0
    A4 = np.array([[.5, .5, 0, 0], [.5, -.5, 0, 0],
                   [0, 0, .5, .5], [0, 0, .5, -.5]], np.float64)
    St = A4.T @ S @ A4
    bt = A4.T @ np.asarray(M, np.float64)[0]
    # LDL^T on the UNSCALED form: St = Lt diag(dl) Lt'. The eta-tile
    # scaling (tiles hold scl_j * v_j) is absorbed into the chain
    # coefficients and the Square scales; values stay well-ranged because
    # tile contents are scl_j*v_j exactly.
    n = 4
    Lt = np.eye(n); dl = np.zeros(n); Sw = St.copy()
    for i in range(n):
        dl[i] = Sw[i, i]
        assert abs(dl[i]) > 1e-14, "singular LDL pivot"
        for j in range(i + 1, n):
            Lt[j, i] = Sw[j, i] / dl[i]
        Sw[i + 1:, i + 1:] -= np.outer(Lt[i + 1:, i], Lt[i + 1:, i]) * dl[i]
    assert np.abs(Lt).max() < 1e4, "ill-conditioned LDL"
    beta = np.linalg.solve(Lt, bt)
    gamma = beta / (2.0 * dl)
    c["phi_const"] = float(mo - (beta * beta / (4.0 * dl)).sum())
    # m~_i = scl_i * m_i built from tiles v~_j = scl_j*v_j:
    #   m~_i = v~_i + sum_{j>i} (scl_i*Lt[j,i]/scl_j) v~_j
    scl = [1.0, 1.0, as_, bs]
    c["l21"] = float(scl[0] * Lt[1, 0] / scl[1])
    c["l31"] = float(scl[0] * Lt[2, 0] / scl[2])
    c["l41"] = float(scl[0] * Lt[3, 0] / scl[3])
    c["l32"] = float(scl[1] * Lt[2, 1] / scl[2])
    c["l42"] = float(scl[1] * Lt[3, 1] / scl[3])
    c["l43"] = float(scl[2] * Lt[3, 2] / scl[3])
    c["sq_scale"] = [float(np.sqrt(abs(dl[i])) / scl[i]) for i in range(4)]
    c["sq_bias"] = [float(np.sqrt(abs(dl[i])) * gamma[i]) for i in range(4)]
    c["sq_sgn"] = [1.0 if d >= 0 else -1.0 for d in dl]
    # global accumulation sign: s.t. first pair is representable
    s0 = c["sq_sgn"][0]
    c["acc_ops"] = [OP.add if sg == s0 else OP.subtract for sg in c["sq_sgn"]]
    c["sig_scale"] = s0

    # --- cost quadratic, complete-squared ---
    ap_c = 0.5 + R_COST * kp * kp
    am_c = 0.5 + R_COST * km * km
    ax_c = 2.0 * R_COST * kp * km
    c["lJ"] = float(ax_c / (2.0 * ap_c))
    c["sqJ1"] = float(np.sqrt(ap_c))
    d2 = am_c - ax_c * ax_c / (4.0 * ap_c)
    assert d2 > 0
    c["sqJ2"] = float(np.sqrt(d2))
    c["term5"] = 5.0
    c["sc0"] = float(2.0 + R_COST * k1 * k1)
    return c


def _build(c, n_chunks=N_CHUNKS):
    n_full = c["n_full"]
    nc = bacc.Bacc("TRN2", target_bir_lowering=False, debug=False,
                   num_devices=N_CORES)
    w_in = nc.dram_tensor("w", [n_chunks * 128, 2, T], F32,
                          kind="ExternalInput").ap()
    j_out = nc.dram_tensor("j", [n_chunks * 128, 1], F32,
                           kind="ExternalOutput").ap()

    ov_bufs = 2 if n_full == 1 else 1
    with tile.TileContext(nc) as tc:
        with (
            tc.tile_pool(name="const", bufs=1) as cpool,
            tc.tile_pool(name="wload", bufs=2) as wpool,
            tc.tile_pool(name="work", bufs=1) as wk,
            tc.tile_pool(name="ov", bufs=ov_bufs) as ov,
            tc.tile_pool(name="sq", bufs=2) as sqpool,
            tc.tile_pool(name="tiny", bufs=2) as tiny,
        ):
            # broadcast-able scalar constants ([128,1], free-step-0 views)
            def const_col(tag, val):
                t = cpool.tile([128, 1], F32, tag=tag)
                nc.gpsimd.memset(t[:, :], val)
                return t

            lamp1 = const_col("lamp1", c["lamp"])
            lamm1 = const_col("lamm1", c["lamm"])
            c1md1 = const_col("c1md1", 1.0 - c["dbar"])
            mu1c = const_col("mu1c", c["mu1"])
            mu2c = const_col("mu2c", c["mu2"])
            ccp_b = const_col("ccp_b", c["ccp"])
            ccm_b = const_col("ccm_b", c["ccm"])
            # sigmoid computes sigma(scale*in + bias): bias applied AFTER
            # the scale, so it carries the unscaled phi constant.
            sigb2 = const_col("sigb2", c["phi_const"])
            sqb = [const_col(f"sqb{i}", c["sq_bias"][i]) for i in range(4)]

            lamp_b = lamp1[:, :].to_broadcast((128, T))
            lamm_b = lamm1[:, :].to_broadcast((128, T))
            c1md_b = c1md1[:, :].to_broadcast((128, T))
            mu1_b = mu1c[:, :].to_broadcast((128, T))
            mu2_b = mu2c[:, :].to_broadcast((128, T))

            for ch in range(n_chunks):
                rows = slice(ch * 128, (ch + 1) * 128)
                W1 = wpool.tile([128, T], F32, tag="W1")
                W2 = wpool.tile([128, T], F32, tag="W2")
                nc.sync.dma_start(W1[:, :], w_in[rows, 0, :])
                nc.sync.dma_start(W2[:, :], w_in[rows, 1, :])

                WPC = ov.tile([128, T], F32, tag="WPC")
                D1M = ov.tile([128, T], F32, tag="D1M")
                nc.gpsimd.tensor_tensor(WPC[:, :], W1[:, :], W2[:, :], OP.add)
                nc.gpsimd.tensor_tensor(D1M[:, :], W1[:, :], W2[:, :], OP.subtract)
                nc.scalar.activation(WPC[:, :], WPC[:, :], AF.Identity,
                                     bias=ccp_b[:, 0:1], scale=1.0)
                nc.scalar.activation(D1M[:, :], D1M[:, :], AF.Identity,
                                     bias=ccm_b[:, 0:1], scale=1.0)

                YM = ov.tile([128, T + 1], F32, tag="YM")
                nc.gpsimd.memset(YM[:, 0:1], 1.0)
                nc.vector.tensor_tensor_scan(
                    YM[:, 1:T + 1], lamm_b, D1M[:, :], 1.0, OP.mult, OP.add)

                # etams = b_u * etam (dbar-gated ym chain, pre-scaled)
                ETAMS = ov.tile([128, T], F32, tag="ETAMS")
                DME = wk.tile([128, T], F32, tag="DME")
                nc.scalar.activation(DME[:, :], YM[:, 0:T], AF.Copy,
                                     scale=c["etams_d1s"])
                nc.vector.tensor_tensor_scan(
                    ETAMS[:, :], c1md_b, DME[:, :], c["etams_init"],
                    OP.mult, OP.add)

                # r = etams + wpc ; z-scans (exact-LTI warm start)
                RR = wk.tile([128, T], F32, tag="RR")
                nc.vector.tensor_tensor(RR[:, :], ETAMS[:, :], WPC[:, :], OP.add)
                Z1 = wk.tile([128, T], F32, tag="Z1")
                Z2 = wk.tile([128, T], F32, tag="Z2")
                DZ1 = wk.tile([128, T], F32, tag="DME")
                nc.scalar.activation(DZ1[:, :], RR[:, :], AF.Copy, scale=c["dz1"])
                nc.vector.tensor_tensor_scan(
                    Z1[:, :], mu1_b, DZ1[:, :], c["z01"], OP.mult, OP.add)
                DZ2 = wk.tile([128, T], F32, tag="DZ2")
                nc.scalar.activation(DZ2[:, :], RR[:, :], AF.Copy, scale=c["dz2"])
                nc.vector.tensor_tensor_scan(
                    Z2[:, :], mu2_b, DZ2[:, :], c["z02"], OP.mult, OP.add)
                # etaps_t (t=0..T-1) = scl_p * (rat_p*Z1 + Z2) at slots 0..T-1
                ETAPS = wk.tile([128, T], F32, tag="ETAPS")
                nc.vector.scalar_tensor_tensor(
                    ETAPS[:, :], Z1[:, :], c["rat_p"], Z2[:, :], OP.mult, OP.add)
                nc.scalar.activation(ETAPS[:, :], ETAPS[:, :], AF.Copy,
                                     scale=c["scl_p"])

                YP = wk.tile([128, T + 1], F32, tag="YP")
                nc.gpsimd.memset(YP[:, 0:1], 1.0)
                D = wk.tile([128, T], F32, tag="D")
                nc.gpsimd.memset(D[:, 0:1], 0.0)
                EB = wk.tile([128, T], F32, tag="EB")
                M1 = wk.tile([128, T - 1], F32, tag="EB")
                M2 = wk.tile([128, T - 1], F32, tag="Z2")
                M3 = wk.tile([128, T - 1], F32, tag="DZ2")
                PH = wk.tile([128, T - 1], F32, tag="Z1")
                AD = tiny.tile([128, 1], F32, tag="AD")
                A1 = tiny.tile([128, 1], F32, tag="A1")
                A2t = tiny.tile([128, 1], F32, tag="A2t")
                TA = tiny.tile([128, 1], F32, tag="TA")
                TB = tiny.tile([128, 1], F32, tag="TB")
                JT = tiny.tile([128, 1], F32, tag="JT")

                for it in range(n_full):
                    final = (it == n_full - 1)
                    # E = WPC + etaps + etams (all pre-scaled)
                    nc.vector.tensor_tensor(EB[:, :], ETAPS[:, :], WPC[:, :], OP.add)
                    nc.vector.tensor_tensor(EB[:, :], ETAMS[:, :], EB[:, :], OP.add)
                    nc.vector.tensor_tensor_scan(
                        YP[:, 1:T + 1], lamp_b, EB[:, :], 1.0, OP.mult, OP.add)

                    # phi via LDL^T on v = (yp_t, ym_t, etaps_{t-1}, etams_{t-1})
                    v1 = YP[:, 1:T]
                    v2 = YM[:, 1:T]
                    v3 = ETAPS[:, 0:T - 1]
                    v4 = ETAMS[:, 0:T - 1]
                    nc.vector.scalar_tensor_tensor(
                        M1[:, :], v2, c["l21"], v1, OP.mult, OP.add)
                    nc.vector.scalar_tensor_tensor(
                        M1[:, :], v3, c["l31"], M1[:, :], OP.mult, OP.add)
                    nc.vector.scalar_tensor_tensor(
                        M1[:, :], v4, c["l41"], M1[:, :], OP.mult, OP.add)
                    nc.vector.scalar_tensor_tensor(
                        M2[:, :], v3, c["l32"], v2, OP.mult, OP.add)
                    nc.vector.scalar_tensor_tensor(
                        M2[:, :], v4, c["l42"], M2[:, :], OP.mult, OP.add)
                    nc.vector.scalar_tensor_tensor(
                        M3[:, :], v4, c["l43"], v3, OP.mult, OP.add)
                    R_ = []
                    for mi, i in ((M1[:, :], 0), (M2[:, :], 1),
                                  (M3[:, :], 2), (v4, 3)):
                        Rt = sqpool.tile([128, T - 1], F32, tag="R")
                        nc.scalar.activation(Rt[:, :], mi, AF.Square,
                                             bias=sqb[i][:, 0:1],
                                             scale=c["sq_scale"][i])
                        R_.append(Rt)
                    nc.vector.tensor_tensor(PH[:, :], R_[0][:, :], R_[1][:, :],
                                            c["acc_ops"][1])
                    nc.vector.tensor_tensor(PH[:, :], PH[:, :], R_[2][:, :],
                                            c["acc_ops"][2])
                    nc.vector.tensor_tensor(PH[:, :], PH[:, :], R_[3][:, :],
                                            c["acc_ops"][3])
                    if final:
                        nc.scalar.activation(D[:, 1:T], PH[:, :], AF.Sigmoid,
                                             bias=sigb2[:, 0:1],
                                             scale=c["sig_scale"],
                                             accum_out=AD[:, :])
                    else:
                        nc.scalar.activation(D[:, 1:T], PH[:, :], AF.Sigmoid,
                                             bias=sigb2[:, 0:1],
                                             scale=c["sig_scale"])
                        # refresh eta scans with time-varying gate
                        DM1 = wk.tile([128, T], F32, tag="DM1x")
                        nc.scalar.activation(DM1[:, :], D[:, :], AF.Identity,
                                             bias=1.0, scale=-1.0)
                        D1P = wk.tile([128, T], F32, tag="DME")
                        YPS = wk.tile([128, T], F32, tag="RR")
                        nc.scalar.activation(YPS[:, :], YP[:, 0:T], AF.Copy,
                                             scale=c["a_u"])
                        nc.gpsimd.tensor_tensor(D1P[:, :], D[:, :], YPS[:, :],
                                                OP.mult)
                        nc.vector.tensor_tensor_scan(
                            ETAPS[:, :], DM1[:, :], D1P[:, :], c["a_u"],
                            OP.mult, OP.add)
                        D1ME = wk.tile([128, T], F32, tag="DZ2")
                        YMS = wk.tile([128, T], F32, tag="YMSx")
                        nc.scalar.activation(YMS[:, :], YM[:, 0:T], AF.Copy,
                                             scale=c["b_u"])
                        nc.gpsimd.tensor_tensor(D1ME[:, :], D[:, :], YMS[:, :],
                                                OP.mult)
                        nc.vector.tensor_tensor_scan(
                            ETAMS[:, :], DM1[:, :], D1ME[:, :], c["b_u"],
                            OP.mult, OP.add)

                # --- cost assembly ---
                yps = YP[:, 1:T - 1]     # t = 1..T-2 (width 2046)
                yms = YM[:, 1:T - 1]
                MJ = M1[:, 0:T - 2]
                SCR = M2[:, 0:T - 2]
                nc.vector.scalar_tensor_tensor(
                    MJ, yms, c["lJ"], yps, OP.mult, OP.add)
                nc.scalar.activation(SCR, MJ, AF.Square, scale=c["sqJ1"],
                                     accum_out=A1[:, :])
                nc.scalar.activation(SCR, yms, AF.Square, scale=c["sqJ2"],
                                     accum_out=A2t[:, :])
                nc.vector.scalar_tensor_tensor(
                    TA[:, :], YP[:, T:T + 1], c["term5"], YP[:, T:T + 1],
                    OP.mult, OP.mult)
                nc.vector.scalar_tensor_tensor(
                    TB[:, :], YM[:, T:T + 1], c["term5"], YM[:, T:T + 1],
                    OP.mult, OP.mult)
                nc.vector.tensor_tensor(JT[:, :], A1[:, :], A2t[:, :], OP.add)
                # AD sums delta over t=1..T-1; cost wants t=1..T-2
                nc.vector.tensor_tensor(JT[:, :], JT[:, :], AD[:, :], OP.add)
                nc.vector.tensor_tensor(JT[:, :], JT[:, :], D[:, T - 1:T],
                                        OP.subtract)
                nc.vector.tensor_tensor(JT[:, :], JT[:, :], TA[:, :], OP.add)
                nc.vector.tensor_tensor(JT[:, :], JT[:, :], TB[:, :], OP.add)
                nc.vector.tensor_scalar(JT[:, :], JT[:, :], c["sc0"], None,
                                        OP.add)
                nc.sync.dma_start(j_out[rows, :], JT[:, :])

    nc.compile()
    return nc


def kernel(w, K, L, M, Mo, trace=False, n_full=1):
    w = np.ascontiguousarray(np.asarray(w, np.float32))
    assert w.shape == (B_TOTAL, 2, T), w.shape
    c = _host_consts(K, L, M, Mo, n_full=n_full)
    nc = _build(c)
    in_maps = [{"w": w[i * B_CORE:(i + 1) * B_CORE]} for i in range(N_CORES)]
    res = bass_utils.run_bass_kernel_spmd(
        nc, in_maps, list(range(N_CORES)), trace=trace)
    out = np.concatenate([res.results[i]["j"].reshape(-1)
                          for i in range(N_CORES)])
    if trace:
        kernel.last_exec_time_ns = res.exec_time_ns
        kernel.last_results = res
    return out.astype(np.float32)


# revision 14
# speedup vs baseline: 3.9263x; 1.0024x over previous
"""Trainium2 Bass kernel for the CSTR gated-estimator trajectory-cost problem.

Reformulation vs the reference's 2048-step sequential scan:

1. The RK4 step with linear plant dynamics is an affine update
       x_{t+1} = P x_t + q*u_t*[1,1] + c + w_t,   P = I + H*A,
   which diagonalizes in the +/- basis yp = x1+x2, ym = x1-x2:
       yp_{t+1} = 0.99*yp_t + (2q*u_t + ccp + wp_t)
       ym_{t+1} = 0.97*ym_t + (ccm + wm_t)        (control-free chain)
   The gated-estimator recurrence stays diagonal in the same basis:
       etap_t = (1-d_t)*etap_{t-1} + d_t*yp_t     (ditto etam/ym)
   with gate d_t = sigmoid(phi_t), phi_t a quadratic form in
   v_t = (yp_t, ym_t, etap_{t-1}, etam_{t-1}).

2. Every recurrence above is a first-order linear scan along time ->
   one hardware `tensor_tensor_scan` instruction per sequence.

3. The nonlinear gate is resolved by one Picard correction on top of an
   EXACT warm start: freezing the gate at dbar = sigmoid(Mo), the coupled
   (yp, etap) system is linear time-invariant; its 2x2 transition matrix
   is eigendecomposed on the host, so the warm-start trajectories
   (including the full control feedback) come from two more scans.
   One full Picard iteration then restores the time-varying gate;
   the residual is below the f32 rounding floor of the reference
   (validated: max rel err ~2.4e-6, identical to 3 iterations).

4. phi is evaluated per time-step via an LDL^T factorization:
   phi = sum_i d_i (m_i + g_i)^2 + const with unit-triangular m-chains
   (6 scalar_tensor_tensor ops), squares+shifts on the Scalar engine
   (Square activation with per-partition bias), and the constant folded
   into the sigmoid bias. The cost quadratic is likewise
   complete-squared so its time-sums come free from Square+accum_out.

Layout: pure data parallel. 8192 samples = 8 cores x 8 chunks x 128
partitions; time (2048) along the free dimension. HBM traffic is w read
exactly once. The only sequential-in-time work is inside scan
instructions; everything else is full-width elementwise DVE/ACT/GPSIMD.
"""

import numpy as np

import concourse.bacc as bacc
import concourse.bass as bass
import concourse.mybir as mybir
import concourse.tile as tile
from concourse import bass_utils

dt = mybir.dt
F32 = dt.float32
AF = mybir.ActivationFunctionType
OP = mybir.AluOpType
AX = mybir.AxisListType

B_TOTAL = 8192
N_CORES = 8
T = 2048
B_CORE = B_TOTAL // N_CORES          # 1024
N_CHUNKS = B_CORE // 128             # 8
H = 0.01
R_COST = 0.1


def _host_consts(K, L, M, Mo, n_full=1):
    """All scalar constants, computed in f64, emitted as f32 immediates."""
    k1 = float(np.asarray(K)[0, 0]); k2 = float(np.asarray(K)[0, 1])
    kp, km = (k1 + k2) / 2.0, (k1 - k2) / 2.0
    c = {"n_full": n_full}
    lamp, lamm = 1.0 - H, 1.0 - 3 * H          # 0.99, 0.97
    c["lamp"], c["lamm"] = lamp, lamm
    c1, c2 = H * H / 2.0, -H * H
    c["ccp"], c["ccm"] = c1 + c2, c1 - c2
    a_u, b_u = 2.0 * H * kp, 2.0 * H * km       # u-feedback coeffs (yp input)
    mo = float(np.asarray(Mo)[0, 0])
    dbar = 1.0 / (1.0 + np.exp(-mo))
    c["mo"], c["dbar"] = mo, dbar

    # --- exact-LTI warm start: s_t = (yp_t, etap_{t-1}) ---
    # Tile conventions (no extra per-element scaling ops anywhere):
    #   tile4 (ETAMS slot) = etam / dbar   (scan d1 = YM directly)
    #   zhat_i = z_i / Vi[i,0]             (scan d1 = RR directly)
    #   tile3 (ETAPS slot) = etap / kap2 = ratq*zhat1 + zhat2 (one STT)
    bs = b_u if abs(b_u) > 1e-12 else 1e-12
    as_ = a_u if abs(a_u) > 1e-12 else 1e-12
    c["a_u"], c["b_u"] = as_, bs
    for jit in range(6):
        db = dbar + (jit * 7e-4 if jit else 0.0)
        A2 = np.array([[lamp + a_u * db, a_u * (1 - db)],
                       [db, 1 - db]], np.float64)
        eigval, V = np.linalg.eig(A2)
        if np.abs(eigval.imag).max() > 1e-12:
            continue
        eigval = eigval.real; V = V.real
        Vi = np.linalg.inv(V)
        kap2 = V[1, 1] * Vi[1, 0]
        if (abs(Vi[0, 0]) < 1e-9 or abs(Vi[1, 0]) < 1e-9
                or abs(kap2) < 1e-9
                or abs(V[1, 1]) < 1e-9):
            continue
        ratq = (V[1, 0] * Vi[0, 0]) / kap2
        if abs(ratq) > 1e5:
            continue
        dbar = db
        break
    else:
        raise AssertionError("degenerate warm-start eigensystem")
    c["dbar"] = dbar
    z0 = Vi @ np.array([1.0, 1.0])
    c["mu1"], c["mu2"] = float(eigval[0]), float(eigval[1])
    c["z01h"] = float(z0[0] / Vi[0, 0])      # zhat1 init
    c["z02h"] = float(z0[1] / Vi[1, 0])      # zhat2 init
    c["ratq"] = float(ratq)
    c["kap2"] = float(kap2)
    # RR = b_u*etam + wpc = (b_u*dbar)*tile4 + WPC
    c["rr_s"] = float(b_u * dbar)
    # E-chain: E = WPC + (a_u*kap2)*tile3 + (b_u*dbar)*tile4
    c["e3_s"] = float(a_u * kap2)
    c["e4_s"] = float(b_u * dbar)
    c["t4_init"] = float(1.0 / dbar)         # etam_{-1}=1 -> tile4 init
    c["t3_init"] = float(1.0 / kap2)         # refresh-scan init (etap_{-1}=1)

    # --- phi quadratic in v = (yp, ym, etaps, etams) (scaled vars) ---
    S = (np.asarray(L, np.float64) + np.asarray(L, np.float64).T) / 2.0
    A4 = np.array([[.5, .5, 0, 0], [.5, -.5, 0, 0],
                   [0, 0, .5, .5], [0, 0, .5, -.5]], np.float64)
    St = A4.T @ S @ A4
    bt = A4.T @ np.asarray(M, np.float64)[0]
    # LDL^T on the UNSCALED form: St = Lt diag(dl) Lt'. The eta-tile
    # scaling (tiles hold scl_j * v_j) is absorbed into the chain
    # coefficients and the Square scales; values stay well-ranged because
    # tile contents are scl_j*v_j exactly.
    n = 4
    Lt = np.eye(n); dl = np.zeros(n); Sw = St.copy()
    for i in range(n):
        dl[i] = Sw[i, i]
        assert abs(dl[i]) > 1e-14, "singular LDL pivot"
        for j in range(i + 1, n):
            Lt[j, i] = Sw[j, i] / dl[i]
        Sw[i + 1:, i + 1:] -= np.outer(Lt[i + 1:, i], Lt[i + 1:, i]) * dl[i]
    assert np.abs(Lt).max() < 1e4, "ill-conditioned LDL"
    beta = np.linalg.solve(Lt, bt)
    gamma = beta / (2.0 * dl)
    c["phi_const"] = float(mo - (beta * beta / (4.0 * dl)).sum())
    # m~_i = scl_i * m_i built from tiles v~_j = scl_j*v_j:
    #   m~_i = v~_i + sum_{j>i} (scl_i*Lt[j,i]/scl_j) v~_j
    scl = [1.0, 1.0, 1.0 / c["kap2"], 1.0 / dbar]
    c["l21"] = float(scl[0] * Lt[1, 0] / scl[1])
    c["l31"] = float(scl[0] * Lt[2, 0] / scl[2])
    c["l41"] = float(scl[0] * Lt[3, 0] / scl[3])
    c["l32"] = float(scl[1] * Lt[2, 1] / scl[2])
    c["l42"] = float(scl[1] * Lt[3, 1] / scl[3])
    c["l43"] = float(scl[2] * Lt[3, 2] / scl[3])
    c["sq_scale"] = [float(np.sqrt(abs(dl[i])) / scl[i]) for i in range(4)]
    c["sq_bias"] = [float(np.sqrt(abs(dl[i])) * gamma[i]) for i in range(4)]
    c["sq_sgn"] = [1.0 if d >= 0 else -1.0 for d in dl]
    # global accumulation sign: s.t. first pair is representable
    s0 = c["sq_sgn"][0]
    c["acc_ops"] = [OP.add if sg == s0 else OP.subtract for sg in c["sq_sgn"]]
    c["sig_scale"] = s0

    # --- cost quadratic, complete-squared ---
    ap_c = 0.5 + R_COST * kp * kp
    am_c = 0.5 + R_COST * km * km
    ax_c = 2.0 * R_COST * kp * km
    c["lJ"] = float(ax_c / (2.0 * ap_c))
    c["sqJ1"] = float(np.sqrt(ap_c))
    d2 = am_c - ax_c * ax_c / (4.0 * ap_c)
    assert d2 > 0
    c["sqJ2"] = float(np.sqrt(d2))
    c["term5"] = 5.0
    c["sc0"] = float(2.0 + R_COST * k1 * k1)
    return c


def _build(c, n_chunks=N_CHUNKS):
    n_full = c["n_full"]
    nc = bacc.Bacc("TRN2", target_bir_lowering=False, debug=False,
                   num_devices=N_CORES)
    w_in = nc.dram_tensor("w", [n_chunks * 128, 2, T], F32,
                          kind="ExternalInput").ap()
    j_out = nc.dram_tensor("j", [n_chunks * 128, 1], F32,
                           kind="ExternalOutput").ap()

    ov_bufs = 2 if n_full == 1 else 1
    with tile.TileContext(nc) as tc:
        with (
            tc.tile_pool(name="const", bufs=1) as cpool,
            tc.tile_pool(name="wload", bufs=2) as wpool,
            tc.tile_pool(name="work", bufs=1) as wk,
            tc.tile_pool(name="ov", bufs=ov_bufs) as ov,
            tc.tile_pool(name="sq", bufs=2) as sqpool,
            tc.tile_pool(name="tiny", bufs=2) as tiny,
        ):
            # broadcast-able scalar constants ([128,1], free-step-0 views)
            def const_col(tag, val):
                t = cpool.tile([128, 1], F32, tag=tag)
                nc.gpsimd.memset(t[:, :], val)
                return t

            lamp1 = const_col("lamp1", c["lamp"])
            lamm1 = const_col("lamm1", c["lamm"])
            c1md1 = const_col("c1md1", 1.0 - c["dbar"])
            mu1c = const_col("mu1c", c["mu1"])
            mu2c = const_col("mu2c", c["mu2"])
            ccp_b = const_col("ccp_b", c["ccp"])
            ccm_b = const_col("ccm_b", c["ccm"])
            # sigmoid computes sigma(scale*in + bias): bias applied AFTER
            # the scale, so it carries the unscaled phi constant.
            sigb2 = const_col("sigb2", c["phi_const"])
            sqb = [const_col(f"sqb{i}", c["sq_bias"][i]) for i in range(4)]

            lamp_b = lamp1[:, :].to_broadcast((128, T))
            lamm_b = lamm1[:, :].to_broadcast((128, T))
            c1md_b = c1md1[:, :].to_broadcast((128, T))
            mu1_b = mu1c[:, :].to_broadcast((128, T))
            mu2_b = mu2c[:, :].to_broadcast((128, T))

            for ch in range(n_chunks):
                rows = slice(ch * 128, (ch + 1) * 128)
                W1 = wpool.tile([128, T], F32, tag="W1")
                W2 = wpool.tile([128, T], F32, tag="W2")
                nc.sync.dma_start(W1[:, :], w_in[rows, 0, :])
                nc.sync.dma_start(W2[:, :], w_in[rows, 1, :])

                WPC = ov.tile([128, T], F32, tag="WPC")
                D1M = ov.tile([128, T], F32, tag="D1M")
                nc.gpsimd.tensor_tensor(WPC[:, :], W1[:, :], W2[:, :], OP.add)
                nc.gpsimd.tensor_tensor(D1M[:, :], W1[:, :], W2[:, :], OP.subtract)
                nc.scalar.activation(WPC[:, :], WPC[:, :], AF.Identity,
                                     bias=ccp_b[:, 0:1], scale=1.0)
                nc.scalar.activation(D1M[:, :], D1M[:, :], AF.Identity,
                                     bias=ccm_b[:, 0:1], scale=1.0)

                YM = ov.tile([128, T + 1], F32, tag="YM")
                nc.gpsimd.memset(YM[:, 0:1], 1.0)
                nc.vector.tensor_tensor_scan(
                    YM[:, 1:T + 1], lamm_b, D1M[:, :], 1.0, OP.mult, OP.add)

                # tile4 = etam/dbar: dbar-gated ym chain, d1 = YM directly
                ETAMS = ov.tile([128, T], F32, tag="ETAMS")
                nc.vector.tensor_tensor_scan(
                    ETAMS[:, :], c1md_b, YM[:, 0:T], c["t4_init"],
                    OP.mult, OP.add)

                # r = b_u*etam + wpc ; zhat-scans (exact-LTI warm start)
                RR = wk.tile([128, T], F32, tag="RR")
                nc.vector.scalar_tensor_tensor(
                    RR[:, :], ETAMS[:, :], c["rr_s"], WPC[:, :], OP.mult, OP.add)
                Z1 = wk.tile([128, T], F32, tag="Z1")
                Z2 = wk.tile([128, T], F32, tag="Z2")
                nc.vector.tensor_tensor_scan(
                    Z1[:, :], mu1_b, RR[:, :], c["z01h"], OP.mult, OP.add)
                nc.vector.tensor_tensor_scan(
                    Z2[:, :], mu2_b, RR[:, :], c["z02h"], OP.mult, OP.add)
                # tile3 = etap/kap2 at slots 0..T-1 = ratq*zhat1 + zhat2
                ETAPS = ov.tile([128, T], F32, tag="ETAPS")
                nc.vector.scalar_tensor_tensor(
                    ETAPS[:, :], Z1[:, :], c["ratq"], Z2[:, :], OP.mult, OP.add)

                YP = ov.tile([128, T + 1], F32, tag="YP")
                nc.gpsimd.memset(YP[:, 0:1], 1.0)
                D = ov.tile([128, T], F32, tag="D")
                nc.gpsimd.memset(D[:, 0:1], 0.0)
                EB = wk.tile([128, T], F32, tag="EB")
                M1 = wk.tile([128, T - 1], F32, tag="EB")
                M2 = wk.tile([128, T - 1], F32, tag="Z2")
                M3 = wk.tile([128, T - 1], F32, tag="RR")
                PH = wk.tile([128, T - 1], F32, tag="Z1")
                AD = tiny.tile([128, 1], F32, tag="AD")
                A1 = tiny.tile([128, 1], F32, tag="A1")
                A2t = tiny.tile([128, 1], F32, tag="A2t")
                TA = tiny.tile([128, 1], F32, tag="TA")
                TB = tiny.tile([128, 1], F32, tag="TB")
                JT = tiny.tile([128, 1], F32, tag="JT")

                for it in range(n_full):
                    final = (it == n_full - 1)
                    # E = WPC + (a_u*kap2)*tile3 + (b_u*dbar)*tile4
                    nc.vector.scalar_tensor_tensor(
                        EB[:, :], ETAPS[:, :], c["e3_s"], WPC[:, :],
                        OP.mult, OP.add)
                    nc.vector.scalar_tensor_tensor(
                        EB[:, :], ETAMS[:, :], c["e4_s"], EB[:, :],
                        OP.mult, OP.add)
                    nc.vector.tensor_tensor_scan(
                        YP[:, 1:T + 1], lamp_b, EB[:, :], 1.0, OP.mult, OP.add)

                    # phi via LDL^T on v = (yp_t, ym_t, etaps_{t-1}, etams_{t-1})
                    v1 = YP[:, 1:T]
                    v2 = YM[:, 1:T]
                    v3 = ETAPS[:, 0:T - 1]
                    v4 = ETAMS[:, 0:T - 1]
                    nc.vector.scalar_tensor_tensor(
                        M1[:, :], v2, c["l21"], v1, OP.mult, OP.add)
                    nc.vector.scalar_tensor_tensor(
                        M1[:, :], v3, c["l31"], M1[:, :], OP.mult, OP.add)
                    nc.vector.scalar_tensor_tensor(
                        M1[:, :], v4, c["l41"], M1[:, :], OP.mult, OP.add)
                    nc.vector.scalar_tensor_tensor(
                        M2[:, :], v3, c["l32"], v2, OP.mult, OP.add)
                    nc.vector.scalar_tensor_tensor(
                        M2[:, :], v4, c["l42"], M2[:, :], OP.mult, OP.add)
                    nc.vector.scalar_tensor_tensor(
                        M3[:, :], v4, c["l43"], v3, OP.mult, OP.add)
                    R_ = []
                    for mi, i in ((M1[:, :], 0), (M2[:, :], 1),
                                  (M3[:, :], 2), (v4, 3)):
                        Rt = sqpool.tile([128, T - 1], F32, tag="R")
                        nc.scalar.activation(Rt[:, :], mi, AF.Square,
                                             bias=sqb[i][:, 0:1],
                                             scale=c["sq_scale"][i])
                        R_.append(Rt)
                    nc.vector.tensor_tensor(PH[:, :], R_[0][:, :], R_[1][:, :],
                                            c["acc_ops"][1])
                    nc.vector.tensor_tensor(PH[:, :], PH[:, :], R_[2][:, :],
                                            c["acc_ops"][2])
                    nc.vector.tensor_tensor(PH[:, :], PH[:, :], R_[3][:, :],
                                            c["acc_ops"][3])
                    if final:
                        nc.scalar.activation(D[:, 1:T], PH[:, :], AF.Sigmoid,
                                             bias=sigb2[:, 0:1],
                                             scale=c["sig_scale"],
                                             accum_out=AD[:, :])
                    else:
                        nc.scalar.activation(D[:, 1:T], PH[:, :], AF.Sigmoid,
                                             bias=sigb2[:, 0:1],
                                             scale=c["sig_scale"])
                        # refresh eta scans with time-varying gate
                        DM1 = wk.tile([128, T], F32, tag="DM1x")
                        nc.scalar.activation(DM1[:, :], D[:, :], AF.Identity,
                                             bias=1.0, scale=-1.0)
                        D1P = wk.tile([128, T], F32, tag="D1Px")
                        YPS = wk.tile([128, T], F32, tag="RR")
                        nc.scalar.activation(YPS[:, :], YP[:, 0:T], AF.Copy,
                                             scale=1.0 / c["kap2"])
                        nc.gpsimd.tensor_tensor(D1P[:, :], D[:, :], YPS[:, :],
                                                OP.mult)
                        ETAPS = ov.tile([128, T], F32, tag="ETAPS")
                        nc.vector.tensor_tensor_scan(
                            ETAPS[:, :], DM1[:, :], D1P[:, :], c["t3_init"],
                            OP.mult, OP.add)
                        D1ME = wk.tile([128, T], F32, tag="D1Px")
                        YMS = wk.tile([128, T], F32, tag="YMSx")
                        nc.scalar.activation(YMS[:, :], YM[:, 0:T], AF.Copy,
                                             scale=1.0 / c["dbar"])
                        nc.gpsimd.tensor_tensor(D1ME[:, :], D[:, :], YMS[:, :],
                                                OP.mult)
                        ETAMS = ov.tile([128, T], F32, tag="ETAMS")
                        nc.vector.tensor_tensor_scan(
                            ETAMS[:, :], DM1[:, :], D1ME[:, :], c["t4_init"],
                            OP.mult, OP.add)

                # --- cost assembly ---
                yps = YP[:, 1:T - 1]     # t = 1..T-2 (width 2046)
                yms = YM[:, 1:T - 1]
                MJ = M1[:, 0:T - 2]
                SCR = M2[:, 0:T - 2]
                nc.vector.scalar_tensor_tensor(
                    MJ, yms, c["lJ"], yps, OP.mult, OP.add)
                nc.scalar.activation(SCR, MJ, AF.Square, scale=c["sqJ1"],
                                     accum_out=A1[:, :])
                nc.scalar.activation(SCR, yms, AF.Square, scale=c["sqJ2"],
                                     accum_out=A2t[:, :])
                nc.vector.scalar_tensor_tensor(
                    TA[:, :], YP[:, T:T + 1], c["term5"], YP[:, T:T + 1],
                    OP.mult, OP.mult)
                nc.vector.scalar_tensor_tensor(
                    TB[:, :], YM[:, T:T + 1], c["term5"], YM[:, T:T + 1],
                    OP.mult, OP.mult)
                nc.vector.tensor_tensor(JT[:, :], A1[:, :], A2t[:, :], OP.add)
                # AD sums delta over t=1..T-1; cost wants t=1..T-2
                nc.vector.tensor_tensor(JT[:, :], JT[:, :], AD[:, :], OP.add)
                nc.vector.tensor_tensor(JT[:, :], JT[:, :], D[:, T - 1:T],
                                        OP.subtract)
                nc.vector.tensor_tensor(JT[:, :], JT[:, :], TA[:, :], OP.add)
                nc.vector.tensor_tensor(JT[:, :], JT[:, :], TB[:, :], OP.add)
                nc.vector.tensor_scalar(JT[:, :], JT[:, :], c["sc0"], None,
                                        OP.add)
                nc.sync.dma_start(j_out[rows, :], JT[:, :])

    nc.compile()
    return nc


def kernel(w, K, L, M, Mo, trace=False, n_full=1):
    w = np.ascontiguousarray(np.asarray(w, np.float32))
    assert w.shape == (B_TOTAL, 2, T), w.shape
    c = _host_consts(K, L, M, Mo, n_full=n_full)
    nc = _build(c)
    in_maps = [{"w": w[i * B_CORE:(i + 1) * B_CORE]} for i in range(N_CORES)]
    res = bass_utils.run_bass_kernel_spmd(
        nc, in_maps, list(range(N_CORES)), trace=trace)
    out = np.concatenate([res.results[i]["j"].reshape(-1)
                          for i in range(N_CORES)])
    if trace:
        kernel.last_exec_time_ns = res.exec_time_ns
        kernel.last_results = res
    return out.astype(np.float32)
